# revision 5
# baseline (speedup 1.0000x reference)
"""Trainium2 Bass kernel for BodyConvClothGraphConvolution.

Reference computation (R = C = 8192, D = H = 256):
    X0  = notes @ w                     # (R+C, H)
    top = X0[:R] + weight @ X0[R:]      # (R, H)
    out = concat([relu(top + b), relu(b)*ones(C,H), X0[R:]], axis=0)

Sharding (8 cores, zero cross-core communication):
  - weight rows and cloth notes rows are sharded 8-way (1024 rows/core).
  - body notes / w / b are replicated; every core recomputes the small
    projected body block X0[R:] (cheap: ~1.2 GFLOP vs 4.3 GFLOP main matmul).
  - SPMD trick: each core sees the 64 body-vertex 128-blocks rotated by
    8*core_id, so the *same* program's "first 8 local blocks" are a distinct
    global slice of X0[R:] on every core -> full X0[R:] gathered on host.

Per-core kernel (all matmuls bf16 inputs, fp32 PSUM accumulation):
  phase 0: ~9 dummy N=512 matmuls on a memset tile warm the PE HAM clock
           gate (~3.4us busy flips 1.2GHz -> 2.4GHz) while input DMA lands
  phase 2: X0b[c,h]  = notes_body @ w      (64 psum tiles, cast to bf16)
  phase 3: topT[h,m] = b + (notes_cloth @ w).T + (weight_shard @ X0b).T
           4 persistent PSUM banks (2 h-tiles x 2 m-chunks of 512); the
           weight streams m-major (all 64 c-blocks for m-chunk 0, then
           m-chunk 1) so the first two banks finish mid-stream and their
           relu+store overlaps the second half of the weight stream.
"""

import numpy as np
import ml_dtypes

R, C, D, H = 8192, 8192, 256, 256
NCORES = 8
MSHARD = R // NCORES          # 1024 cloth rows per core
NMT = MSHARD // 128           # 8 output row-tiles per core
NCT = C // 128                # 64 body-vertex 128-blocks
NDT = D // 128                # 2 contraction tiles for notes @ w
OWN = NCT // NCORES           # 8 body blocks output per core
NHT = H // 128                # 2 h-tiles (transposed phase-3 output)
NSLAB = 8                     # weight DMA slabs per m-chunk pass (1MB each)
N_WARM = 9                    # HAM warmup dummy matmuls

BF16 = ml_dtypes.bfloat16

_CACHE = {}


def _build_nc(reps=1, loop_iters=1):
    """Build + compile the SPMD Bass program (same program for all cores).

    reps > 1 statically repeats the whole body; loop_iters > 1 wraps the body
    in a hardware For_i loop. Both are used only by the timing harness to
    isolate per-execution device time by wall-clock slope.
    """
    import concourse.bass as bass
    import concourse.bacc as bacc
    import concourse.tile as tile
    from concourse import mybir

    fp32 = mybir.dt.float32
    bf16 = mybir.dt.bfloat16

    nc = bacc.Bacc("TRN2", target_bir_lowering=False, debug=False,
                   num_devices=NCORES)

    # DRAM I/O (per-core shapes)
    nbt_d = nc.dram_tensor("nbt", [NDT, 4, 128, 2048], bf16,
                           kind="ExternalInput").ap()
    nct_d = nc.dram_tensor("nct", [128, NDT * MSHARD], bf16,
                           kind="ExternalInput").ap()
    wt_d = nc.dram_tensor("wt", [128, NDT * H], bf16,
                          kind="ExternalInput").ap()
    b2_d = nc.dram_tensor("b2", [128, NHT], fp32, kind="ExternalInput").ap()
    wpe_d = nc.dram_tensor("wpe", [2 * NSLAB, 128, 8 * 512], bf16,
                           kind="ExternalInput").ap()
    top_d = nc.dram_tensor("topt_out", [NHT, 128, MSHARD], fp32,
                           kind="ExternalOutput").ap()
    x0b_d = nc.dram_tensor("x0b_out", [OWN, 128, H], fp32,
                           kind="ExternalOutput").ap()

    def body(tc, const_pool, wpe_pool, ps2_pool, ps3_pool, out_pool):
        # ---- phase 0: HAM warmup (no deps at all; PE busy from ~0.1us) ----
        # Reads an uninitialized SBUF tile on purpose: the products are never
        # consumed, the point is ~3.4us of PE activity so the HAM clock gate
        # opens (1.2 -> 2.4 GHz) before the first real matmul.
        warm_sb = const_pool.tile([128, 512], bf16)
        # shares the phase-2 PSUM slot group ("ps" tag); released before
        # phase 2 needs all four banks
        warm_ps = ps2_pool.tile([128, 512], fp32, name="warm", tag="ps")
        for i in range(N_WARM):
            nc.tensor.matmul(warm_ps[:, :], lhsT=warm_sb[:, 0:128],
                             rhs=warm_sb[:, :], start=True, stop=True)

        wt_sb = const_pool.tile([128, NDT * H], bf16)
        nct_sb = const_pool.tile([128, NDT * MSHARD], bf16)
        b2_sb = const_pool.tile([128, NHT], fp32)
        nbt_sb = const_pool.tile([128, NDT * C], bf16)
        x0b_bf = const_pool.tile([128, NCT * H], bf16)

        nc.sync.dma_start(out=wt_sb[:, :], in_=wt_d[:, :])
        nc.sync.dma_start(out=b2_sb[:, :], in_=b2_d[:, :])
        # tiny first chunk so the very first phase-2 matmul unblocks ~2us
        # earlier, then fine chunks in (dt0, dt1) pairs
        for dt in range(NDT):
            nc.sync.dma_start(out=nbt_sb[:, dt * C:dt * C + 128],
                              in_=nbt_d[dt, 0, :, 0:128])
        for cc in range(8):
            lo = 128 if cc == 0 else 0
            for dt in range(NDT):
                nc.sync.dma_start(
                    out=nbt_sb[:, dt * C + cc * 1024 + lo:
                               dt * C + (cc + 1) * 1024],
                    in_=nbt_d[dt, cc // 2, :,
                              (cc % 2) * 1024 + lo:(cc % 2 + 1) * 1024],
                )
        nc.sync.dma_start(out=nct_sb[:, :], in_=nct_d[:, :])

        # ---- phase 2: X0b = notes_body @ w, tile by tile ----
        x0b_stage = []
        for ct in range(NCT):
            ps = ps2_pool.tile([128, H], fp32)
            for dt in range(NDT):
                nc.tensor.matmul(
                    ps[:, :],
                    lhsT=nbt_sb[:, dt * C + ct * 128:dt * C + (ct + 1) * 128],
                    rhs=wt_sb[:, dt * H:(dt + 1) * H],
                    start=(dt == 0),
                    stop=(dt == NDT - 1),
                )
            # bf16 copy feeds the big matmul; alternate DVE/ACT so PSUM
            # slot recycling isn't gated by a single engine's copy rate
            if ct % 2 == 0 or ct < OWN:
                nc.vector.tensor_copy(out=x0b_bf[:, ct * H:(ct + 1) * H],
                                      in_=ps[:, :])
            else:
                nc.scalar.copy(out=x0b_bf[:, ct * H:(ct + 1) * H],
                               in_=ps[:, :])
            if ct < OWN:
                # stage in SBUF now; DMA to HBM deferred past the
                # bandwidth-critical head window
                o = const_pool.tile([128, H], fp32, name=f"x0bst{ct}",
                                    tag=f"x0bst{ct}")
                nc.scalar.copy(out=o[:, :], in_=ps[:, :])
                x0b_stage.append(o)

        # ---- phase 3 (transposed): topT[h, m] = b + X0cT + (W @ X0b).T ----
        # psg bank (ht, mc) accumulates topT[ht*128:(ht+1)*128,
        # mc*512:(mc+1)*512]; bias b varies along PSUM partitions here, so it
        # fuses into the ACT relu as a per-partition bias.
        psg = [ps3_pool.tile([128, 512], fp32, name=f"psg{g}", tag=f"psg{g}")
               for g in range(NHT * 2)]
        for ht in range(NHT):
            for mc in range(2):
                for dt in range(NDT):
                    nc.tensor.matmul(
                        psg[ht * 2 + mc][:, :],
                        lhsT=wt_sb[:, dt * H + ht * 128:dt * H + (ht + 1) * 128],
                        rhs=nct_sb[:, dt * MSHARD + mc * 512:
                                   dt * MSHARD + (mc + 1) * 512],
                        start=(dt == 0), stop=False,
                    )
        # m-major weight stream: all 64 c-blocks for m-chunk 0 (slabs 0..7),
        # then for m-chunk 1 (slabs 8..15). Banks (*, mc) finish at the end
        # of pass mc; pass-0 relu+store overlaps pass 1.
        for mc in range(2):
            for cbp in range(NSLAB):
                wslab = wpe_pool.tile([128, 8 * 512], bf16)
                nc.sync.dma_start(out=wslab[:, :], in_=wpe_d[mc * NSLAB + cbp])
                if mc == 0 and cbp == 4:
                    # deferred X0b block stores: the head DMA crunch is over
                    for ct, o in enumerate(x0b_stage):
                        nc.sync.dma_start(out=x0b_d[ct], in_=o[:, :])
                    x0b_stage = []
                for j in range(8):
                    ct = cbp * 8 + j
                    for ht in range(NHT):
                        nc.tensor.matmul(
                            psg[ht * 2 + mc][:, :],
                            lhsT=x0b_bf[:, ct * H + ht * 128:
                                        ct * H + (ht + 1) * 128],
                            rhs=wslab[:, j * 512:(j + 1) * 512],
                            start=False, stop=(ct == NCT - 1),
                        )
            for ht in range(NHT):
                o = out_pool.tile([128, 512], fp32, tag="topout")
                nc.scalar.activation(o[:, :], psg[ht * 2 + mc][:, :],
                                     mybir.ActivationFunctionType.Relu,
                                     bias=b2_sb[:, ht:ht + 1])
                nc.sync.dma_start(out=top_d[ht, :, mc * 512:(mc + 1) * 512],
                                  in_=o[:, :])

    with tile.TileContext(nc) as tc:
        with (
            tc.tile_pool(name="const", bufs=1) as const_pool,
            tc.tile_pool(name="wpe", bufs=4) as wpe_pool,
            tc.tile_pool(name="ps2", bufs=4, space="PSUM") as ps2_pool,
            tc.tile_pool(name="ps3", bufs=1, space="PSUM") as ps3_pool,
            tc.tile_pool(name="outs", bufs=4) as out_pool,
        ):
            pools = (const_pool, wpe_pool, ps2_pool, ps3_pool, out_pool)
            if loop_iters > 1:
                with tc.For_i(0, loop_iters, 1,
                              hint_engines=(mybir.EngineType.PE,)):
                    body(tc, *pools)
            else:
                for _rep in range(reps):
                    body(tc, *pools)

    nc.compile()
    return nc


def _get_nc(reps=1, loop_iters=1):
    key = ("nc", reps, loop_iters)
    if key not in _CACHE:
        _CACHE[key] = _build_nc(reps, loop_iters)
    return _CACHE[key]


def _pack_inputs(notes, weight, w, b):
    """Host-side shard + transpose + bf16 cast into per-core in_maps."""
    nb = np.ascontiguousarray(notes[R:]).astype(BF16)      # (C, D)
    ncl = np.ascontiguousarray(notes[:R]).astype(BF16)     # (R, D)
    wq = w.astype(BF16)                                    # (D, H)
    nbT = np.ascontiguousarray(nb.T)                       # (D, C)

    wt = np.ascontiguousarray(
        wq.reshape(NDT, 128, H).transpose(1, 0, 2).reshape(128, NDT * H))
    b2 = np.ascontiguousarray(b.reshape(NHT, 128).T)       # (128, NHT) f32

    base = np.arange(C)
    in_maps = []
    for k in range(NCORES):
        # local column x -> global body column perm[x] (blocks rotated by 8k)
        perm = ((base // 128 + OWN * k) % NCT) * 128 + base % 128

        nbt = nbT[:, perm]                                  # (D, C)
        nbt = np.ascontiguousarray(
            nbt.reshape(NDT, 128, 4, 2048).transpose(0, 2, 1, 3))

        nck = ncl[k * MSHARD:(k + 1) * MSHARD]              # (MSHARD, D)
        nct = np.ascontiguousarray(
            nck.T.reshape(NDT, 128, MSHARD).transpose(1, 0, 2)
            .reshape(128, NDT * MSHARD))

        wk = weight[k * MSHARD:(k + 1) * MSHARD].astype(BF16)   # (MSHARD, C)
        wkp = wk[:, perm]
        # [mc*8+cbp, p(c_local), j*512+n] = wkp[mc*512+n, (cbp*8+j)*128+p]
        wpe = np.ascontiguousarray(
            wkp.reshape(2, 512, NSLAB, 8, 128).transpose(0, 2, 4, 3, 1)
            .reshape(2 * NSLAB, 128, 8 * 512))

        in_maps.append({
            "nbt": nbt, "nct": nct, "wt": wt, "b2": b2, "wpe": wpe,
        })
    return in_maps


def kernel(notes, weight, w, b):
    from concourse.bass_utils import run_bass_kernel_spmd

    notes = np.asarray(notes, dtype=np.float32)
    weight = np.asarray(weight, dtype=np.float32)
    w = np.asarray(w, dtype=np.float32)
    b = np.asarray(b, dtype=np.float32)

    nc = _get_nc()
    in_maps = _pack_inputs(notes, weight, w, b)
    res = run_bass_kernel_spmd(nc, in_maps, core_ids=list(range(NCORES)),
                               trace=False)

    out = np.empty((R + 2 * C, H), dtype=np.float32)
    for k in range(NCORES):
        r = res.results[k]
        out[k * MSHARD:(k + 1) * MSHARD] = \
            r["topt_out"].reshape(H, MSHARD).T
        out[R + C + k * MSHARD:R + C + (k + 1) * MSHARD] = \
            r["x0b_out"].reshape(MSHARD, H)
    out[R:R + C] = np.maximum(b, 0.0)[None, :]
    return out


# revision 21
# speedup vs baseline: 1.0155x; 1.0155x over previous
"""Trainium2 Bass kernel for BodyConvClothGraphConvolution.

Reference computation (R = C = 8192, D = H = 256):
    X0  = notes @ w                     # (R+C, H)
    top = X0[:R] + weight @ X0[R:]      # (R, H)
    out = concat([relu(top + b), relu(b)*ones(C,H), X0[R:]], axis=0)

Sharding (8 cores, zero cross-core communication):
  - weight rows and cloth notes rows are sharded 8-way (1024 rows/core).
  - body notes / w / b are replicated; every core recomputes the small
    projected body block X0[R:] (cheap: ~1.2 GFLOP vs 4.3 GFLOP main matmul).
  - SPMD trick: each core sees the 64 body-vertex 128-blocks rotated by
    8*core_id, so the *same* program's "first 8 local blocks" are a distinct
    global slice of X0[R:] on every core -> full X0[R:] gathered on host.

Per-core kernel (all matmuls bf16 inputs, fp32 PSUM accumulation):
  phase 0: ~9 dummy N=512 matmuls on a memset tile warm the PE HAM clock
           gate (~3.4us busy flips 1.2GHz -> 2.4GHz) while input DMA lands
  phase 2: X0b[c,h]  = notes_body @ w      (64 psum tiles, cast to bf16)
  phase 3: topT[h,m] = b + (notes_cloth @ w).T + (weight_shard @ X0b).T
           4 persistent PSUM banks (2 h-tiles x 2 m-chunks of 512); the
           weight streams m-major (all 64 c-blocks for m-chunk 0, then
           m-chunk 1) so the first two banks finish mid-stream and their
           relu+store overlaps the second half of the weight stream.
"""

import numpy as np
import ml_dtypes

R, C, D, H = 8192, 8192, 256, 256
NCORES = 8
MSHARD = R // NCORES          # 1024 cloth rows per core
NMT = MSHARD // 128           # 8 output row-tiles per core
NCT = C // 128                # 64 body-vertex 128-blocks
NDT = D // 128                # 2 contraction tiles for notes @ w
OWN = NCT // NCORES           # 8 body blocks output per core
NHT = H // 128                # 2 h-tiles (transposed phase-3 output)
NSLAB = 8                     # weight DMA slabs per m-chunk pass (1MB each)
N_WARM = 9                    # HAM warmup dummy matmuls

BF16 = ml_dtypes.bfloat16

_CACHE = {}


def _build_nc(reps=1, loop_iters=1):
    """Build + compile the SPMD Bass program (same program for all cores).

    reps > 1 statically repeats the whole body; loop_iters > 1 wraps the body
    in a hardware For_i loop. Both are used only by the timing harness to
    isolate per-execution device time by wall-clock slope.
    """
    import concourse.bass as bass
    import concourse.bacc as bacc
    import concourse.tile as tile
    from concourse import mybir

    fp32 = mybir.dt.float32
    bf16 = mybir.dt.bfloat16

    nc = bacc.Bacc("TRN2", target_bir_lowering=False, debug=False,
                   num_devices=NCORES)

    # DRAM I/O (per-core shapes)
    nbt_d = nc.dram_tensor("nbt", [NDT, 4, 128, 2048], bf16,
                           kind="ExternalInput").ap()
    nct_d = nc.dram_tensor("nct", [128, NDT * MSHARD], bf16,
                           kind="ExternalInput").ap()
    wt_d = nc.dram_tensor("wt", [128, NDT * H], bf16,
                          kind="ExternalInput").ap()
    b2_d = nc.dram_tensor("b2", [128, NHT], fp32, kind="ExternalInput").ap()
    wpe_d = nc.dram_tensor("wpe", [2 * NSLAB, 128, 8 * 512], bf16,
                           kind="ExternalInput").ap()
    top_d = nc.dram_tensor("topt_out", [NHT, 128, MSHARD], fp32,
                           kind="ExternalOutput").ap()
    x0b_d = nc.dram_tensor("x0b_out", [128, OWN * H], bf16,
                           kind="ExternalOutput").ap()

    def body(tc, const_pool, wpe_pool, ps2_pool, ps3_pool, out_pool):
        # ---- phase 0: HAM warmup (PE busy from ~0.1us) ----
        # The matmuls read columns the memset never touches (subtile dep
        # tracking -> no cross-engine wait); the memset only exists so the
        # tile has a writer and can be allocated/released. Garbage inputs are
        # fine: the products are never consumed, the point is ~3.4us of PE
        # activity so the HAM clock gate opens (1.2 -> 2.4 GHz) before the
        # first real matmul.
        warm_sb = const_pool.tile([128, 644], bf16)
        nc.vector.memset(warm_sb[:, 0:4], 0.0)
        # shares the phase-2 PSUM slot group ("ps" tag); released before
        # phase 2 needs all four banks
        warm_ps = ps2_pool.tile([128, 512], fp32, name="warm", tag="ps")
        for i in range(N_WARM):
            nc.tensor.matmul(warm_ps[:, :], lhsT=warm_sb[:, 4:132],
                             rhs=warm_sb[:, 132:644], start=True, stop=True)

        wt_sb = const_pool.tile([128, NDT * H], bf16)
        nct_sb = const_pool.tile([128, NDT * MSHARD], bf16)
        b2_sb = const_pool.tile([128, NHT], fp32)
        nbt_sb = const_pool.tile([128, NDT * C], bf16)
        x0b_bf = const_pool.tile([128, NCT * H], bf16)

        nc.sync.dma_start(out=wt_sb[:, :], in_=wt_d[:, :])
        # b2 is only needed by the relus ~50us in; keep it off the critical
        # preamble (each HWDGE slot delays the nbt stream ~0.65us) by
        # issuing it on the scalar-engine HWDGE queue
        nc.scalar.dma_start(out=b2_sb[:, :], in_=b2_d[:, :])
        # tiny first chunk so the very first phase-2 matmul unblocks ~2us
        # earlier, then fine chunks in (dt0, dt1) pairs
        for dt in range(NDT):
            nc.sync.dma_start(out=nbt_sb[:, dt * C:dt * C + 128],
                              in_=nbt_d[dt, 0, :, 0:128])
        for cc in range(8):
            lo = 128 if cc == 0 else 0
            for dt in range(NDT):
                nc.sync.dma_start(
                    out=nbt_sb[:, dt * C + cc * 1024 + lo:
                               dt * C + (cc + 1) * 1024],
                    in_=nbt_d[dt, cc // 2, :,
                              (cc % 2) * 1024 + lo:(cc % 2 + 1) * 1024],
                )
        nc.sync.dma_start(out=nct_sb[:, :], in_=nct_d[:, :])

        # ---- phase 2: X0b = notes_body @ w, two 128-blocks per PSUM bank ----
        # Two c-blocks share one PSUM tile (bank): the copy-out slot-recycle
        # loop (copy + completion-sem back to PE) is ~1.7us; 4 banks x 2
        # blocks of matmul work keeps PE ahead of that latency. The second
        # block's start=True clears has_written bits bank-wide but leaves the
        # first block's data intact (bits only gate accumulate-vs-overwrite).
        for cp in range(NCT // 2):
            ps = ps2_pool.tile([128, 2 * H], fp32)
            for half in range(2):
                ct = 2 * cp + half
                for dt in range(NDT):
                    nc.tensor.matmul(
                        ps[:, half * H:(half + 1) * H],
                        lhsT=nbt_sb[:, dt * C + ct * 128:
                                    dt * C + (ct + 1) * 128],
                        rhs=wt_sb[:, dt * H:(dt + 1) * H],
                        start=(dt == 0),
                        stop=(dt == NDT - 1),
                    )
            # bf16 copy feeds the big matmul; alternate DVE/ACT so PSUM
            # slot recycling isn't gated by a single engine's copy rate
            if cp % 2 == 0:
                nc.vector.tensor_copy(out=x0b_bf[:, cp * 2 * H:
                                                 (cp + 1) * 2 * H],
                                      in_=ps[:, :])
            else:
                nc.scalar.copy(out=x0b_bf[:, cp * 2 * H:(cp + 1) * 2 * H],
                               in_=ps[:, :])

        # ---- phase 3 (transposed): topT[h, m] = b + X0cT + (W @ X0b).T ----
        # psg bank (ht, mc) accumulates topT[ht*128:(ht+1)*128,
        # mc*512:(mc+1)*512]; bias b varies along PSUM partitions here, so it
        # fuses into the ACT relu as a per-partition bias.
        psg = [ps3_pool.tile([128, 512], fp32, name=f"psg{g}", tag=f"psg{g}")
               for g in range(NHT * 2)]
        for ht in range(NHT):
            for mc in range(2):
                for dt in range(NDT):
                    nc.tensor.matmul(
                        psg[ht * 2 + mc][:, :],
                        lhsT=wt_sb[:, dt * H + ht * 128:dt * H + (ht + 1) * 128],
                        rhs=nct_sb[:, dt * MSHARD + mc * 512:
                                   dt * MSHARD + (mc + 1) * 512],
                        start=(dt == 0), stop=False,
                    )
        # m-major weight stream: all 64 c-blocks for m-chunk 0 (slabs 0..7),
        # then for m-chunk 1 (slabs 8..15). Banks (*, mc) finish at the end
        # of pass mc; pass-0 relu+store overlaps pass 1.
        for mc in range(2):
            for cbp in range(NSLAB):
                wslab = wpe_pool.tile([128, 8 * 512], bf16)
                if mc == 0 and cbp == 0:
                    # split the first fetch so phase-3 main unblocks on the
                    # first half-slab ~1.5us earlier (subtile deps)
                    nc.sync.dma_start(out=wslab[:, 0:4 * 512],
                                      in_=wpe_d[0, :, 0:4 * 512])
                    nc.sync.dma_start(out=wslab[:, 4 * 512:8 * 512],
                                      in_=wpe_d[0, :, 4 * 512:8 * 512])
                else:
                    nc.sync.dma_start(out=wslab[:, :],
                                      in_=wpe_d[mc * NSLAB + cbp])
                if mc == 0 and cbp == 4:
                    # deferred X0b block store (head DMA crunch is over):
                    # one contiguous bf16 burst from x0b_bf on the
                    # sync-engine HWDGE FIFO, so it queues behind this
                    # slab's fetch instead of being hoisted into the input
                    # stream; host upcasts + reshapes
                    nc.sync.dma_start(out=x0b_d[:, :],
                                      in_=x0b_bf[:, 0:OWN * H])
                for j in range(8):
                    ct = cbp * 8 + j
                    for ht in range(NHT):
                        nc.tensor.matmul(
                            psg[ht * 2 + mc][:, :],
                            lhsT=x0b_bf[:, ct * H + ht * 128:
                                        ct * H + (ht + 1) * 128],
                            rhs=wslab[:, j * 512:(j + 1) * 512],
                            start=False, stop=(ct == NCT - 1),
                        )
            # relu in 256-wide halves (ACT starts on the first-finished
            # bank early), one batched store per h-tile; the two tail
            # stores go to different HWDGE queues so they don't serialize
            for ht in range(NHT):
                o = out_pool.tile([128, 512], fp32, tag="topout")
                for hf in range(2):
                    nc.scalar.activation(
                        o[:, hf * 256:(hf + 1) * 256],
                        psg[ht * 2 + mc][:, hf * 256:(hf + 1) * 256],
                        mybir.ActivationFunctionType.Relu,
                        bias=b2_sb[:, ht:ht + 1])
                eng = nc.sync if ht == 0 else nc.scalar
                eng.dma_start(
                    out=top_d[ht, :, mc * 512:(mc + 1) * 512],
                    in_=o[:, :])

    with tile.TileContext(nc) as tc:
        with (
            tc.tile_pool(name="const", bufs=1) as const_pool,
            tc.tile_pool(name="wpe", bufs=6) as wpe_pool,
            tc.tile_pool(name="ps2", bufs=4, space="PSUM") as ps2_pool,
            tc.tile_pool(name="ps3", bufs=1, space="PSUM") as ps3_pool,
            tc.tile_pool(name="outs", bufs=4) as out_pool,
        ):
            pools = (const_pool, wpe_pool, ps2_pool, ps3_pool, out_pool)
            if loop_iters > 1:
                with tc.For_i(0, loop_iters, 1,
                              hint_engines=(mybir.EngineType.PE,)):
                    body(tc, *pools)
            else:
                for _rep in range(reps):
                    body(tc, *pools)

    nc.compile()
    return nc


def _get_nc(reps=1, loop_iters=1):
    key = ("nc", reps, loop_iters)
    if key not in _CACHE:
        _CACHE[key] = _build_nc(reps, loop_iters)
    return _CACHE[key]


def _pack_inputs(notes, weight, w, b):
    """Host-side shard + transpose + bf16 cast into per-core in_maps."""
    nb = np.ascontiguousarray(notes[R:]).astype(BF16)      # (C, D)
    ncl = np.ascontiguousarray(notes[:R]).astype(BF16)     # (R, D)
    wq = w.astype(BF16)                                    # (D, H)
    nbT = np.ascontiguousarray(nb.T)                       # (D, C)

    wt = np.ascontiguousarray(
        wq.reshape(NDT, 128, H).transpose(1, 0, 2).reshape(128, NDT * H))
    b2 = np.ascontiguousarray(b.reshape(NHT, 128).T)       # (128, NHT) f32

    base = np.arange(C)
    in_maps = []
    for k in range(NCORES):
        # local column x -> global body column perm[x] (blocks rotated by 8k)
        perm = ((base // 128 + OWN * k) % NCT) * 128 + base % 128

        nbt = nbT[:, perm]                                  # (D, C)
        nbt = np.ascontiguousarray(
            nbt.reshape(NDT, 128, 4, 2048).transpose(0, 2, 1, 3))

        nck = ncl[k * MSHARD:(k + 1) * MSHARD]              # (MSHARD, D)
        nct = np.ascontiguousarray(
            nck.T.reshape(NDT, 128, MSHARD).transpose(1, 0, 2)
            .reshape(128, NDT * MSHARD))

        wk = weight[k * MSHARD:(k + 1) * MSHARD].astype(BF16)   # (MSHARD, C)
        wkp = wk[:, perm]
        # [mc*8+cbp, p(c_local), j*512+n] = wkp[mc*512+n, (cbp*8+j)*128+p]
        wpe = np.ascontiguousarray(
            wkp.reshape(2, 512, NSLAB, 8, 128).transpose(0, 2, 4, 3, 1)
            .reshape(2 * NSLAB, 128, 8 * 512))

        in_maps.append({
            "nbt": nbt, "nct": nct, "wt": wt, "b2": b2, "wpe": wpe,
        })
    return in_maps


def kernel(notes, weight, w, b):
    from concourse.bass_utils import run_bass_kernel_spmd

    notes = np.asarray(notes, dtype=np.float32)
    weight = np.asarray(weight, dtype=np.float32)
    w = np.asarray(w, dtype=np.float32)
    b = np.asarray(b, dtype=np.float32)

    nc = _get_nc()
    in_maps = _pack_inputs(notes, weight, w, b)
    res = run_bass_kernel_spmd(nc, in_maps, core_ids=list(range(NCORES)),
                               trace=False)

    out = np.empty((R + 2 * C, H), dtype=np.float32)
    for k in range(NCORES):
        r = res.results[k]
        out[k * MSHARD:(k + 1) * MSHARD] = \
            r["topt_out"].reshape(H, MSHARD).T
        out[R + C + k * MSHARD:R + C + (k + 1) * MSHARD] = \
            r["x0b_out"].reshape(128, OWN, H).transpose(1, 0, 2) \
            .reshape(MSHARD, H).astype(np.float32)
    out[R:R + C] = np.maximum(b, 0.0)[None, :]
    return out


# revision 28
# speedup vs baseline: 1.1438x; 1.1263x over previous
"""Trainium2 Bass kernel for BodyConvClothGraphConvolution.

Reference computation (R = C = 8192, D = H = 256):
    X0  = notes @ w                     # (R+C, H)
    top = X0[:R] + weight @ X0[R:]      # (R, H)
    out = concat([relu(top + b), relu(b)*ones(C,H), X0[R:]], axis=0)

Sharding (8 cores, zero cross-core communication):
  - weight rows and cloth notes rows are sharded 8-way (1024 rows/core);
    body notes NB and w/b are replicated.
  - Associativity: per core, W_k @ (NB @ w) is computed as (W_k @ NB) @ w.
    The 8192-deep contraction runs against raw NB (same FLOPs), but the
    h-projection then applies to the small (1024 x 256) partial P = W_k@NB
    instead of the (8192 x 256) X0b — this removes phase 2's replicated
    projection almost entirely. Each core also projects only its OWN 1024
    body rows for the X0[R:] output block.

Per-core kernel (all matmuls bf16 inputs, fp32 PSUM accumulation; on HW the
sustained matmul rate is ~(131ns + N/2.4GHz)/instruction, so everything is
structured as few, wide N=512 matmuls):
  warm:  ~9 dummy matmuls warm the PE HAM clock gate while input DMA lands
  own:   X0ownT[h, c_own] = (NB_own @ w).T   (8 MMs, w stationary)
  main:  PT[d, m] = (W_k @ NB).T: weight streams m-major (all 64 c-blocks
         for m-chunk 0, then m-chunk 1) through 4 persistent PSUM banks
         (2 d-tiles x 2 m-chunks); NB tiles are the stationary operand.
  fold:  topT[h, m] = b + (Xc0 @ w).T + (P @ w).T — 16 MMs on the small P,
         pass-0's fold/relu/store overlap pass 1 of the weight stream.
"""

import numpy as np
import ml_dtypes

R, C, D, H = 8192, 8192, 256, 256
NCORES = 8
MSHARD = R // NCORES          # 1024 cloth rows / weight rows per core
NCT = C // 128                # 64 body-vertex 128-blocks
NDT = D // 128                # 2 contraction tiles over d
NHT = H // 128                # 2 h-tiles
NSLAB = 8                     # weight DMA slabs per m-chunk pass (1MB each)
N_WARM = 9                    # HAM warmup dummy matmuls

BF16 = ml_dtypes.bfloat16

_CACHE = {}


def _build_nc(reps=1, loop_iters=1):
    """Build + compile the SPMD Bass program (same program for all cores).

    reps > 1 statically repeats the whole body; loop_iters > 1 wraps the body
    in a hardware For_i loop. Both are used only by the timing harness to
    isolate per-execution device time by wall-clock slope.
    """
    import concourse.bass as bass
    import concourse.bacc as bacc
    import concourse.tile as tile
    from concourse import mybir

    fp32 = mybir.dt.float32
    bf16 = mybir.dt.bfloat16

    nc = bacc.Bacc("TRN2", target_bir_lowering=False, debug=False,
                   num_devices=NCORES)

    # DRAM I/O (per-core shapes)
    nb_d = nc.dram_tensor("nb", [128, NCT * D], bf16,
                          kind="ExternalInput").ap()
    nbo_d = nc.dram_tensor("nbo", [128, NDT * MSHARD], bf16,
                           kind="ExternalInput").ap()
    nct_d = nc.dram_tensor("nct", [128, NDT * MSHARD], bf16,
                           kind="ExternalInput").ap()
    wt_d = nc.dram_tensor("wt", [128, NDT * H], bf16,
                          kind="ExternalInput").ap()
    b2_d = nc.dram_tensor("b2", [128, NHT], fp32, kind="ExternalInput").ap()
    wpe_d = nc.dram_tensor("wpe", [2 * NSLAB, 128, 8 * 512], bf16,
                           kind="ExternalInput").ap()
    top_d = nc.dram_tensor("topt_out", [NHT, 128, MSHARD], fp32,
                           kind="ExternalOutput").ap()
    x0b_d = nc.dram_tensor("x0b_out", [128, NHT * MSHARD], bf16,
                           kind="ExternalOutput").ap()

    def body(tc, const_pool, wpe_pool, ps2_pool, ps3_pool, out_pool):
        # ---- HAM warmup (PE busy from ~0.1us) ----
        # The matmuls read columns the memset never touches (subtile dep
        # tracking -> no cross-engine wait); garbage inputs are fine, the
        # products are never consumed. ~3.4us of PE activity opens the HAM
        # clock gate (1.2 -> 2.4 GHz) before the first real matmul.
        warm_sb = const_pool.tile([128, 644], bf16)
        nc.vector.memset(warm_sb[:, 0:4], 0.0)
        warm_ps = ps2_pool.tile([128, 512], fp32, name="warm", tag="ps")
        for i in range(N_WARM):
            nc.tensor.matmul(warm_ps[:, :], lhsT=warm_sb[:, 4:132],
                             rhs=warm_sb[:, 132:644], start=True, stop=True)

        wt_sb = const_pool.tile([128, NDT * H], bf16)
        nbo_sb = const_pool.tile([128, NDT * MSHARD], bf16)
        nb_sb = const_pool.tile([128, NCT * D], bf16)
        nct_sb = const_pool.tile([128, NDT * MSHARD], bf16)
        b2_sb = const_pool.tile([128, NHT], fp32)
        pbf_sb = const_pool.tile([128, NDT * MSHARD], bf16)
        x0o_sb = const_pool.tile([128, NHT * MSHARD], bf16)

        nc.sync.dma_start(out=wt_sb[:, :], in_=wt_d[:, :])
        nc.sync.dma_start(out=nbo_sb[:, :], in_=nbo_d[:, :])
        # b2 off the critical preamble, on the scalar HWDGE queue
        nc.scalar.dma_start(out=b2_sb[:, :], in_=b2_d[:, :])
        # NB streams in 8 chunks of 8 c-blocks (0.5MB each) INTERLEAVED with
        # the pass-0 weight slabs: slab s consumes exactly NB chunk s, so
        # this order lets the main stream start ~11us earlier than
        # NB-then-slabs would
        slabs0 = []
        for cc in range(NSLAB):
            nc.sync.dma_start(
                out=nb_sb[:, cc * 8 * D:(cc + 1) * 8 * D],
                in_=nb_d[:, cc * 8 * D:(cc + 1) * 8 * D])
            wslab = wpe_pool.tile([128, 8 * 512], bf16)
            if cc == 0:
                # split the first fetch so the stream unblocks on the
                # first half-slab earlier (subtile deps)
                nc.sync.dma_start(out=wslab[:, 0:4 * 512],
                                  in_=wpe_d[0, :, 0:4 * 512])
                nc.sync.dma_start(out=wslab[:, 4 * 512:8 * 512],
                                  in_=wpe_d[0, :, 4 * 512:8 * 512])
            else:
                nc.sync.dma_start(out=wslab[:, :], in_=wpe_d[cc])
            slabs0.append(wslab)
            if cc == 4:
                # deferred X0 own-block store (head DMA crunch is over):
                # one contiguous bf16 burst; host upcasts + reshapes
                nc.sync.dma_start(out=x0b_d[:, :], in_=x0o_sb[:, :])

        # ---- own-block projection: X0ownT[h, c_own] = (NB_own @ w).T ----
        for ht in range(NHT):
            for oc in range(2):
                ps = ps2_pool.tile([128, 512], fp32, name="own", tag="ps")
                for dt in range(NDT):
                    nc.tensor.matmul(
                        ps[:, :],
                        lhsT=wt_sb[:, dt * H + ht * 128:
                                   dt * H + (ht + 1) * 128],
                        rhs=nbo_sb[:, dt * MSHARD + oc * 512:
                                   dt * MSHARD + (oc + 1) * 512],
                        start=(dt == 0), stop=(dt == NDT - 1),
                    )
                dst = x0o_sb[:, ht * MSHARD + oc * 512:
                             ht * MSHARD + (oc + 1) * 512]
                if (ht * 2 + oc) % 2 == 0:
                    nc.vector.tensor_copy(out=dst, in_=ps[:, :])
                else:
                    nc.scalar.copy(out=dst, in_=ps[:, :])

        # nct feeds the fold stage (~mid-stream); stream it after NB
        nc.sync.dma_start(out=nct_sb[:, :], in_=nct_d[:, :])

        # ---- main: PT[d, m] = (W_k @ NB).T, m-major weight stream ----
        # PSUM bank (dt, mc) accumulates PT[dt*128:(dt+1)*128,
        # mc*512:(mc+1)*512] over all 64 c-blocks; NB block tiles are the
        # stationary operand, the weight slab the N=512 moving operand.
        pT = [ps3_pool.tile([128, 512], fp32, name=f"pT{g}", tag=f"pT{g}")
              for g in range(NDT * 2)]

        psg = {}

        def fold_nct(mc):
            # init topT[ht][mc] accumulators with the cloth term (Xc0 @ w).T
            # — independent of P, so pass 1's init can run early
            psg[mc] = [ps2_pool.tile([128, 512], fp32, name=f"psg{mc}{ht}",
                                     tag="ps") for ht in range(NHT)]
            for dt in range(NDT):
                for ht in range(NHT):
                    nc.tensor.matmul(
                        psg[mc][ht][:, :],
                        lhsT=wt_sb[:, dt * H + ht * 128:
                                   dt * H + (ht + 1) * 128],
                        rhs=nct_sb[:, dt * MSHARD + mc * 512:
                                   dt * MSHARD + (mc + 1) * 512],
                        start=(dt == 0), stop=False,
                    )

        def fold_mc(mc):
            # += (P @ w).T, then relu(+bias) and store
            for dt in range(NDT):
                for ht in range(NHT):
                    nc.tensor.matmul(
                        psg[mc][ht][:, :],
                        lhsT=wt_sb[:, dt * H + ht * 128:
                                   dt * H + (ht + 1) * 128],
                        rhs=pbf_sb[:, dt * MSHARD + mc * 512:
                                   dt * MSHARD + (mc + 1) * 512],
                        start=False, stop=(dt == NDT - 1),
                    )
            # relu in 256-wide halves, one batched store per h-tile; the
            # two tail stores use different HWDGE queues
            for ht in range(NHT):
                o = out_pool.tile([128, 512], fp32, tag="topout")
                for hf in range(2):
                    nc.scalar.activation(
                        o[:, hf * 256:(hf + 1) * 256],
                        psg[mc][ht][:, hf * 256:(hf + 1) * 256],
                        mybir.ActivationFunctionType.Relu,
                        bias=b2_sb[:, ht:ht + 1])
                eng = nc.sync if ht == 0 else nc.scalar
                eng.dma_start(
                    out=top_d[ht, :, mc * 512:(mc + 1) * 512],
                    in_=o[:, :])

        for mc in range(2):
            for cbp in range(NSLAB):
                if mc == 0:
                    wslab = slabs0[cbp]
                else:
                    wslab = wpe_pool.tile([128, 8 * 512], bf16)
                    nc.sync.dma_start(out=wslab[:, :],
                                      in_=wpe_d[mc * NSLAB + cbp])
                if mc == 1 and cbp == 2:
                    # pass-0 P is copied out by now: fold + relu + store
                    # m-chunk 0 while pass 1 streams; also init pass-1's
                    # accumulators with their (P-independent) cloth term so
                    # the kernel tail is just 4 matmuls + relu + store
                    fold_nct(0)
                    fold_mc(0)
                    fold_nct(1)
                for j in range(8):
                    ct = cbp * 8 + j
                    for dt in range(NDT):
                        nc.tensor.matmul(
                            pT[dt * 2 + mc][:, :],
                            lhsT=nb_sb[:, ct * D + dt * 128:
                                       ct * D + (dt + 1) * 128],
                            rhs=wslab[:, j * 512:(j + 1) * 512],
                            start=(ct == 0), stop=(ct == NCT - 1),
                        )
            # end of pass: copy this pass's PT banks to bf16 for the fold
            for dt in range(NDT):
                dst = pbf_sb[:, dt * MSHARD + mc * 512:
                             dt * MSHARD + (mc + 1) * 512]
                if dt == 0:
                    nc.vector.tensor_copy(out=dst, in_=pT[dt * 2 + mc][:, :])
                else:
                    nc.scalar.copy(out=dst, in_=pT[dt * 2 + mc][:, :])
        fold_mc(1)

    with tile.TileContext(nc) as tc:
        with (
            tc.tile_pool(name="const", bufs=1) as const_pool,
            tc.tile_pool(name="wpe", bufs=8) as wpe_pool,
            tc.tile_pool(name="ps2", bufs=4, space="PSUM") as ps2_pool,
            tc.tile_pool(name="ps3", bufs=1, space="PSUM") as ps3_pool,
            tc.tile_pool(name="outs", bufs=4) as out_pool,
        ):
            pools = (const_pool, wpe_pool, ps2_pool, ps3_pool, out_pool)
            if loop_iters > 1:
                with tc.For_i(0, loop_iters, 1,
                              hint_engines=(mybir.EngineType.PE,)):
                    body(tc, *pools)
            else:
                for _rep in range(reps):
                    body(tc, *pools)

    nc.compile()
    return nc


def _get_nc(reps=1, loop_iters=1):
    key = ("nc", reps, loop_iters)
    if key not in _CACHE:
        _CACHE[key] = _build_nc(reps, loop_iters)
    return _CACHE[key]


def _pack_inputs(notes, weight, w, b):
    """Host-side shard + layout + bf16 cast into per-core in_maps."""
    nb = np.ascontiguousarray(notes[R:]).astype(BF16)      # (C, D)
    ncl = np.ascontiguousarray(notes[:R]).astype(BF16)     # (R, D)
    wq = w.astype(BF16)                                    # (D, H)
    nbT = np.ascontiguousarray(nb.T)                       # (D, C)

    # NB natural blocks: nb_sb[p, ct*D + d] = NB[ct*128 + p, d]
    nbp = np.ascontiguousarray(
        nb.reshape(NCT, 128, D).transpose(1, 0, 2).reshape(128, NCT * D))
    wt = np.ascontiguousarray(
        wq.reshape(NDT, 128, H).transpose(1, 0, 2).reshape(128, NDT * H))
    b2 = np.ascontiguousarray(b.reshape(NHT, 128).T)       # (128, NHT) f32

    in_maps = []
    for k in range(NCORES):
        # own body slice (transposed): nbo[p, dt*MSHARD + c] =
        #   NB[k*MSHARD + c, dt*128 + p]
        nbo = np.ascontiguousarray(
            nbT[:, k * MSHARD:(k + 1) * MSHARD]
            .reshape(NDT, 128, MSHARD).transpose(1, 0, 2)
            .reshape(128, NDT * MSHARD))

        nck = ncl[k * MSHARD:(k + 1) * MSHARD]              # (MSHARD, D)
        nct = np.ascontiguousarray(
            nck.T.reshape(NDT, 128, MSHARD).transpose(1, 0, 2)
            .reshape(128, NDT * MSHARD))

        wk = weight[k * MSHARD:(k + 1) * MSHARD].astype(BF16)   # (MSHARD, C)
        # [mc*8+cbp, p(c_local), j*512+n] = wk[mc*512+n, (cbp*8+j)*128+p]
        wpe = np.ascontiguousarray(
            wk.reshape(2, 512, NSLAB, 8, 128).transpose(0, 2, 4, 3, 1)
            .reshape(2 * NSLAB, 128, 8 * 512))

        in_maps.append({
            "nb": nbp, "nbo": nbo, "nct": nct, "wt": wt, "b2": b2,
            "wpe": wpe,
        })
    return in_maps


def kernel(notes, weight, w, b):
    from concourse.bass_utils import run_bass_kernel_spmd

    notes = np.asarray(notes, dtype=np.float32)
    weight = np.asarray(weight, dtype=np.float32)
    w = np.asarray(w, dtype=np.float32)
    b = np.asarray(b, dtype=np.float32)

    nc = _get_nc()
    in_maps = _pack_inputs(notes, weight, w, b)
    res = run_bass_kernel_spmd(nc, in_maps, core_ids=list(range(NCORES)),
                               trace=False)

    out = np.empty((R + 2 * C, H), dtype=np.float32)
    for k in range(NCORES):
        r = res.results[k]
        out[k * MSHARD:(k + 1) * MSHARD] = \
            r["topt_out"].reshape(H, MSHARD).T
        # x0b_out[p, ht*MSHARD + c] = X0own[c, ht*128 + p]
        out[R + C + k * MSHARD:R + C + (k + 1) * MSHARD] = \
            r["x0b_out"].reshape(128, NHT, MSHARD).transpose(2, 1, 0) \
            .reshape(MSHARD, H).astype(np.float32)
    out[R:R + C] = np.maximum(b, 0.0)[None, :]
    return out


# revision 32
# speedup vs baseline: 1.1832x; 1.0345x over previous
"""Trainium2 Bass kernel for BodyConvClothGraphConvolution.

Reference computation (R = C = 8192, D = H = 256):
    X0  = notes @ w                     # (R+C, H)
    top = X0[:R] + weight @ X0[R:]      # (R, H)
    out = concat([relu(top + b), relu(b)*ones(C,H), X0[R:]], axis=0)

Sharding (8 cores, zero cross-core communication):
  - weight rows and cloth notes rows are sharded 8-way (1024 rows/core);
    body notes NB and w/b are replicated.
  - Associativity: per core, W_k @ (NB @ w) is computed as (W_k @ NB) @ w.
    The 8192-deep contraction runs against raw NB (same FLOPs), but the
    h-projection then applies to the small (1024 x 256) partial P = W_k@NB
    instead of the (8192 x 256) X0b — this removes phase 2's replicated
    projection almost entirely. Each core also projects only its OWN 1024
    body rows for the X0[R:] output block.

Per-core kernel (all matmuls bf16 inputs, fp32 PSUM accumulation; on HW the
sustained matmul rate is ~(131ns + N/2.4GHz)/instruction, so everything is
structured as few, wide N=512 matmuls):
  warm:  ~9 dummy matmuls warm the PE HAM clock gate while input DMA lands
  own:   X0ownT[h, c_own] = (NB_own @ w).T   (8 MMs, w stationary)
  main:  PT[d, m] = (W_k @ NB).T: weight streams m-major (all 64 c-blocks
         for m-chunk 0, then m-chunk 1) through 4 persistent PSUM banks
         (2 d-tiles x 2 m-chunks); NB tiles are the stationary operand.
  fold:  topT[h, m] = b + (Xc0 @ w).T + (P @ w).T — 16 MMs on the small P,
         pass-0's fold/relu/store overlap pass 1 of the weight stream.
"""

import numpy as np
import ml_dtypes

R, C, D, H = 8192, 8192, 256, 256
NCORES = 8
MSHARD = R // NCORES          # 1024 cloth rows / weight rows per core
NCT = C // 128                # 64 body-vertex 128-blocks
NDT = D // 128                # 2 contraction tiles over d
NHT = H // 128                # 2 h-tiles
NSLAB = 8                     # weight DMA slabs per m-chunk pass (1MB each)
N_WARM = 9                    # HAM warmup dummy matmuls

BF16 = ml_dtypes.bfloat16

_CACHE = {}


def _build_nc(reps=1, loop_iters=1):
    """Build + compile the SPMD Bass program (same program for all cores).

    reps > 1 statically repeats the whole body; loop_iters > 1 wraps the body
    in a hardware For_i loop. Both are used only by the timing harness to
    isolate per-execution device time by wall-clock slope.
    """
    import concourse.bass as bass
    import concourse.bacc as bacc
    import concourse.tile as tile
    from concourse import mybir

    fp32 = mybir.dt.float32
    bf16 = mybir.dt.bfloat16

    nc = bacc.Bacc("TRN2", target_bir_lowering=False, debug=False,
                   num_devices=NCORES)

    # DRAM I/O (per-core shapes)
    nb_d = nc.dram_tensor("nb", [128, NCT * D], bf16,
                          kind="ExternalInput").ap()
    nbo_d = nc.dram_tensor("nbo", [128, NDT * MSHARD], bf16,
                           kind="ExternalInput").ap()
    nct_d = nc.dram_tensor("nct", [128, NDT * MSHARD], bf16,
                           kind="ExternalInput").ap()
    wt_d = nc.dram_tensor("wt", [128, NDT * H], bf16,
                          kind="ExternalInput").ap()
    b2_d = nc.dram_tensor("b2", [128, NHT], fp32, kind="ExternalInput").ap()
    wpe_d = nc.dram_tensor("wpe", [2 * NSLAB, 128, 8 * 512], bf16,
                           kind="ExternalInput").ap()
    top_d = nc.dram_tensor("topt_out", [NHT, 128, MSHARD], fp32,
                           kind="ExternalOutput").ap()
    x0b_d = nc.dram_tensor("x0b_out", [128, NHT * MSHARD], bf16,
                           kind="ExternalOutput").ap()

    def body(tc, const_pool, wpe_pool, ps2_pool, ps3_pool, out_pool):
        # ---- HAM warmup (PE busy from ~0.1us) ----
        # The matmuls read columns the memset never touches (subtile dep
        # tracking -> no cross-engine wait); garbage inputs are fine, the
        # products are never consumed. ~3.4us of PE activity opens the HAM
        # clock gate (1.2 -> 2.4 GHz) before the first real matmul.
        warm_sb = const_pool.tile([128, 644], bf16)
        nc.vector.memset(warm_sb[:, 0:4], 0.0)
        warm_ps = ps2_pool.tile([128, 512], fp32, name="warm", tag="ps")
        for i in range(N_WARM):
            nc.tensor.matmul(warm_ps[:, :], lhsT=warm_sb[:, 4:132],
                             rhs=warm_sb[:, 132:644], start=True, stop=True)

        wt_sb = const_pool.tile([128, NDT * H], bf16)
        nbo_sb = const_pool.tile([128, NDT * MSHARD], bf16)
        nb_sb = const_pool.tile([128, NCT * D], bf16)
        nct_sb = const_pool.tile([128, NDT * MSHARD], bf16)
        b2_sb = const_pool.tile([128, NHT], fp32)
        pbf_sb = const_pool.tile([128, NDT * MSHARD], bf16)
        x0o_sb = const_pool.tile([128, NHT * MSHARD], bf16)

        nc.sync.dma_start(out=wt_sb[:, :], in_=wt_d[:, :])
        nc.sync.dma_start(out=nbo_sb[:, :], in_=nbo_d[:, :])
        # b2 off the critical preamble, on the scalar HWDGE queue
        nc.scalar.dma_start(out=b2_sb[:, :], in_=b2_d[:, :])
        # NB streams in 8 chunks of 8 c-blocks (0.5MB each) INTERLEAVED with
        # the pass-0 weight slabs: slab s consumes exactly NB chunk s, so
        # this order lets the main stream start ~11us earlier than
        # NB-then-slabs would
        slabs0 = []
        for cc in range(NSLAB):
            nc.sync.dma_start(
                out=nb_sb[:, cc * 8 * D:(cc + 1) * 8 * D],
                in_=nb_d[:, cc * 8 * D:(cc + 1) * 8 * D])
            wslab = wpe_pool.tile([128, 8 * 512], bf16)
            if cc == 0:
                # split the first fetch so the stream unblocks on the
                # first half-slab earlier (subtile deps)
                nc.sync.dma_start(out=wslab[:, 0:4 * 512],
                                  in_=wpe_d[0, :, 0:4 * 512])
                nc.sync.dma_start(out=wslab[:, 4 * 512:8 * 512],
                                  in_=wpe_d[0, :, 4 * 512:8 * 512])
            else:
                nc.sync.dma_start(out=wslab[:, :], in_=wpe_d[cc])
            slabs0.append(wslab)

        # ---- own-block projection: X0ownT[h, c_own] = (NB_own @ w).T ----
        for ht in range(NHT):
            for oc in range(2):
                ps = ps2_pool.tile([128, 512], fp32, name="own", tag="ps")
                for dt in range(NDT):
                    nc.tensor.matmul(
                        ps[:, :],
                        lhsT=wt_sb[:, dt * H + ht * 128:
                                   dt * H + (ht + 1) * 128],
                        rhs=nbo_sb[:, dt * MSHARD + oc * 512:
                                   dt * MSHARD + (oc + 1) * 512],
                        start=(dt == 0), stop=(dt == NDT - 1),
                    )
                dst = x0o_sb[:, ht * MSHARD + oc * 512:
                             ht * MSHARD + (oc + 1) * 512]
                if (ht * 2 + oc) % 2 == 0:
                    nc.vector.tensor_copy(out=dst, in_=ps[:, :])
                else:
                    nc.scalar.copy(out=dst, in_=ps[:, :])

        # X0 own-block store: one contiguous bf16 burst; host upcasts +
        # reshapes. Issued AFTER the projection writes x0o_sb (program order
        # is dep order for Tile) but queued on the sync FIFO behind the
        # pass-0 slab fetches, so it executes past the head DMA crunch.
        nc.sync.dma_start(out=x0b_d[:, :], in_=x0o_sb[:, :])
        # nct feeds the fold stage (~mid-stream); stream it after NB
        nc.sync.dma_start(out=nct_sb[:, :], in_=nct_d[:, :])

        # ---- main: PT[d, m] = (W_k @ NB).T, m-major weight stream ----
        # PSUM bank (dt, mc) accumulates PT[dt*128:(dt+1)*128,
        # mc*512:(mc+1)*512] over all 64 c-blocks; NB block tiles are the
        # stationary operand, the weight slab the N=512 moving operand.
        pT = [ps3_pool.tile([128, 512], fp32, name=f"pT{g}", tag=f"pT{g}")
              for g in range(NDT * 2)]

        psg = {}

        def fold_nct(mc):
            # init topT[ht][mc] accumulators with the cloth term (Xc0 @ w).T
            # — independent of P, so pass 1's init can run early
            psg[mc] = [ps2_pool.tile([128, 512], fp32, name=f"psg{mc}{ht}",
                                     tag="ps") for ht in range(NHT)]
            for dt in range(NDT):
                for ht in range(NHT):
                    nc.tensor.matmul(
                        psg[mc][ht][:, :],
                        lhsT=wt_sb[:, dt * H + ht * 128:
                                   dt * H + (ht + 1) * 128],
                        rhs=nct_sb[:, dt * MSHARD + mc * 512:
                                   dt * MSHARD + (mc + 1) * 512],
                        start=(dt == 0), stop=False,
                    )

        def fold_mc(mc):
            # += (P @ w).T, then relu(+bias) and store
            for dt in range(NDT):
                for ht in range(NHT):
                    nc.tensor.matmul(
                        psg[mc][ht][:, :],
                        lhsT=wt_sb[:, dt * H + ht * 128:
                                   dt * H + (ht + 1) * 128],
                        rhs=pbf_sb[:, dt * MSHARD + mc * 512:
                                   dt * MSHARD + (mc + 1) * 512],
                        start=False, stop=(dt == NDT - 1),
                    )
            # relu in 256-wide halves, one batched store per h-tile; the
            # two tail stores use different HWDGE queues
            for ht in range(NHT):
                o = out_pool.tile([128, 512], fp32, tag="topout")
                for hf in range(2):
                    nc.scalar.activation(
                        o[:, hf * 256:(hf + 1) * 256],
                        psg[mc][ht][:, hf * 256:(hf + 1) * 256],
                        mybir.ActivationFunctionType.Relu,
                        bias=b2_sb[:, ht:ht + 1])
                eng = nc.sync if ht == 0 else nc.scalar
                eng.dma_start(
                    out=top_d[ht, :, mc * 512:(mc + 1) * 512],
                    in_=o[:, :])

        for mc in range(2):
            for cbp in range(NSLAB):
                if mc == 0:
                    wslab = slabs0[cbp]
                else:
                    wslab = wpe_pool.tile([128, 8 * 512], bf16)
                    nc.sync.dma_start(out=wslab[:, :],
                                      in_=wpe_d[mc * NSLAB + cbp])
                if mc == 1 and cbp == 2:
                    # pass-0 P is copied out by now: fold + relu + store
                    # m-chunk 0 while pass 1 streams
                    fold_nct(0)
                    fold_mc(0)
                for j in range(8):
                    ct = cbp * 8 + j
                    for dt in range(NDT):
                        nc.tensor.matmul(
                            pT[dt * 2 + mc][:, :],
                            lhsT=nb_sb[:, ct * D + dt * 128:
                                       ct * D + (dt + 1) * 128],
                            rhs=wslab[:, j * 512:(j + 1) * 512],
                            start=(ct == 0), stop=(ct == NCT - 1),
                        )
            # end of pass: copy this pass's PT banks to bf16 for the fold
            for dt in range(NDT):
                dst = pbf_sb[:, dt * MSHARD + mc * 512:
                             dt * MSHARD + (mc + 1) * 512]
                if dt == 0:
                    nc.vector.tensor_copy(out=dst, in_=pT[dt * 2 + mc][:, :])
                else:
                    nc.scalar.copy(out=dst, in_=pT[dt * 2 + mc][:, :])
        fold_nct(1)
        fold_mc(1)

    with tile.TileContext(nc) as tc:
        with (
            tc.tile_pool(name="const", bufs=1) as const_pool,
            tc.tile_pool(name="wpe", bufs=8) as wpe_pool,
            tc.tile_pool(name="ps2", bufs=4, space="PSUM") as ps2_pool,
            tc.tile_pool(name="ps3", bufs=1, space="PSUM") as ps3_pool,
            tc.tile_pool(name="outs", bufs=4) as out_pool,
        ):
            pools = (const_pool, wpe_pool, ps2_pool, ps3_pool, out_pool)
            if loop_iters > 1:
                with tc.For_i(0, loop_iters, 1,
                              hint_engines=(mybir.EngineType.PE,)):
                    body(tc, *pools)
            else:
                for _rep in range(reps):
                    body(tc, *pools)

    nc.compile()
    return nc


def _get_nc(reps=1, loop_iters=1):
    key = ("nc", reps, loop_iters)
    if key not in _CACHE:
        _CACHE[key] = _build_nc(reps, loop_iters)
    return _CACHE[key]


def _pack_inputs(notes, weight, w, b):
    """Host-side shard + layout + bf16 cast into per-core in_maps."""
    nb = np.ascontiguousarray(notes[R:]).astype(BF16)      # (C, D)
    ncl = np.ascontiguousarray(notes[:R]).astype(BF16)     # (R, D)
    wq = w.astype(BF16)                                    # (D, H)
    nbT = np.ascontiguousarray(nb.T)                       # (D, C)

    # NB natural blocks: nb_sb[p, ct*D + d] = NB[ct*128 + p, d]
    nbp = np.ascontiguousarray(
        nb.reshape(NCT, 128, D).transpose(1, 0, 2).reshape(128, NCT * D))
    wt = np.ascontiguousarray(
        wq.reshape(NDT, 128, H).transpose(1, 0, 2).reshape(128, NDT * H))
    b2 = np.ascontiguousarray(b.reshape(NHT, 128).T)       # (128, NHT) f32

    in_maps = []
    for k in range(NCORES):
        # own body slice (transposed): nbo[p, dt*MSHARD + c] =
        #   NB[k*MSHARD + c, dt*128 + p]
        nbo = np.ascontiguousarray(
            nbT[:, k * MSHARD:(k + 1) * MSHARD]
            .reshape(NDT, 128, MSHARD).transpose(1, 0, 2)
            .reshape(128, NDT * MSHARD))

        nck = ncl[k * MSHARD:(k + 1) * MSHARD]              # (MSHARD, D)
        nct = np.ascontiguousarray(
            nck.T.reshape(NDT, 128, MSHARD).transpose(1, 0, 2)
            .reshape(128, NDT * MSHARD))

        wk = weight[k * MSHARD:(k + 1) * MSHARD].astype(BF16)   # (MSHARD, C)
        # [mc*8+cbp, p(c_local), j*512+n] = wk[mc*512+n, (cbp*8+j)*128+p]
        wpe = np.ascontiguousarray(
            wk.reshape(2, 512, NSLAB, 8, 128).transpose(0, 2, 4, 3, 1)
            .reshape(2 * NSLAB, 128, 8 * 512))

        in_maps.append({
            "nb": nbp, "nbo": nbo, "nct": nct, "wt": wt, "b2": b2,
            "wpe": wpe,
        })
    return in_maps


def kernel(notes, weight, w, b):
    from concourse.bass_utils import run_bass_kernel_spmd

    notes = np.asarray(notes, dtype=np.float32)
    weight = np.asarray(weight, dtype=np.float32)
    w = np.asarray(w, dtype=np.float32)
    b = np.asarray(b, dtype=np.float32)

    nc = _get_nc()
    in_maps = _pack_inputs(notes, weight, w, b)
    res = run_bass_kernel_spmd(nc, in_maps, core_ids=list(range(NCORES)),
                               trace=False)

    out = np.empty((R + 2 * C, H), dtype=np.float32)
    for k in range(NCORES):
        r = res.results[k]
        out[k * MSHARD:(k + 1) * MSHARD] = \
            r["topt_out"].reshape(H, MSHARD).T
        # x0b_out[p, ht*MSHARD + c] = X0own[c, ht*128 + p]
        out[R + C + k * MSHARD:R + C + (k + 1) * MSHARD] = \
            r["x0b_out"].reshape(128, NHT, MSHARD).transpose(2, 1, 0) \
            .reshape(MSHARD, H).astype(np.float32)
    out[R:R + C] = np.maximum(b, 0.0)[None, :]
    return out
